# revision 4
# baseline (speedup 1.0000x reference)
"""Trainium2 Bass kernel for a single-step RNN cell + softmax projection.

    h_new = tanh(x @ W_ih.T + b_ih + hidden @ W_hh.T + b_hh)   [N, 256]
    out   = softmax(h_new @ W_proj.T + b_proj, axis=1)          [N, 12]

Strategy: pure data parallel over the batch (N=524288) across 8 NeuronCores.
On-chip everything is computed in transposed orientation [feature, batch]
so the batch is the matmul moving dimension (512-column tiles):

    h_preT = W_cat.T^T @ actT       actT = [hidden.T ; x.T ; ones] (281 rows)
    (the ones row x b_h row folds both biases into the accumulation)
    h_newT = tanh(h_preT)                              -> stored transposed
    logitT = W_proj.T^T @ h_newT   (+ b_proj via Exp's per-partition bias)
    expT   = exp(logitT + b_proj)
    den    = ones.T @ expT          (partition-dim reduction via PE)
    outT   = expT * pbcast(1/den)   (partition broadcast on GpSimd)

Matmuls run in float32r (full-rate fp32 on the PE). The host transposes
inputs while sharding and transposes outputs while gathering; all device
DMA transfers are fully contiguous.
"""

import numpy as np

import concourse.bacc as bacc
import concourse.bass as bass
import concourse.mybir as mybir
import concourse.tile as tile
from concourse.bass_utils import run_bass_kernel_spmd

F32 = mybir.dt.float32
F32R = mybir.dt.float32r

N = 524288
IN = 24
H = 256
NOPS = 12
NCORES = 8
NLOC = N // NCORES          # 65536 per core
TILE = 512                  # batch columns per compute tile
NTILES = NLOC // TILE       # 128
KC = H + IN + 1             # 281 contraction rows (hiddenT ; xT ; ones)

Tanh = mybir.ActivationFunctionType.Tanh
Exp = mybir.ActivationFunctionType.Exp

TRACE = False
LAST_RESULTS = None
_LAST_IN_MAPS = None

_NC_CACHE = None


def _build_nc():
    nc = bacc.Bacc("TRN2", target_bir_lowering=False, debug=False,
                   num_devices=NCORES)

    actT = nc.dram_tensor("actT", [KC, NLOC], F32R, kind="ExternalInput")
    wcatT = nc.dram_tensor("wcatT", [KC, H], F32R, kind="ExternalInput")
    wpT = nc.dram_tensor("wpT", [H, NOPS], F32R, kind="ExternalInput")
    bp = nc.dram_tensor("bp", [NOPS, 1], F32, kind="ExternalInput")
    ones12 = nc.dram_tensor("ones12", [NOPS, NOPS], F32R, kind="ExternalInput")
    houtT = nc.dram_tensor("houtT", [H, NLOC], F32R, kind="ExternalOutput")
    ooutT = nc.dram_tensor("ooutT", [NOPS, NLOC], F32, kind="ExternalOutput")

    with tile.TileContext(nc) as tc:
        with (
            tc.tile_pool(name="weights", bufs=1) as wpool,
            tc.tile_pool(name="acts", bufs=4) as apool,
            tc.tile_pool(name="hnew", bufs=3) as hpool,
            tc.tile_pool(name="soft", bufs=3) as spool,
            tc.tile_pool(name="psum_h", bufs=2, space="PSUM") as ph,
            tc.tile_pool(name="psum_l", bufs=2, space="PSUM") as pl,
            tc.tile_pool(name="psum_s", bufs=2, space="PSUM") as ps,
        ):
            wc0 = wpool.tile([128, H], F32R, tag="wc0")
            nc.sync.dma_start(wc0[:], wcatT[0:128, :])
            wc1 = wpool.tile([128, H], F32R, tag="wc1")
            nc.sync.dma_start(wc1[:], wcatT[128:256, :])
            wc2 = wpool.tile([KC - 256, H], F32R, tag="wc2")
            nc.sync.dma_start(wc2[:], wcatT[256:KC, :])
            wp0 = wpool.tile([128, NOPS], F32R, tag="wp0")
            nc.sync.dma_start(wp0[:], wpT[0:128, :])
            wp1 = wpool.tile([128, NOPS], F32R, tag="wp1")
            nc.sync.dma_start(wp1[:], wpT[128:256, :])
            bpt = wpool.tile([NOPS, 1], F32, tag="bpt")
            nc.sync.dma_start(bpt[:], bp[:])
            ones = wpool.tile([NOPS, NOPS], F32R, tag="ones")
            nc.sync.dma_start(ones[:], ones12[:])

            wcs = [wc0, wc1, wc2]
            for i in range(NTILES):
                j0 = i * TILE
                a0 = apool.tile([128, TILE], F32R, tag="a0")
                nc.sync.dma_start(a0[:], actT[0:128, j0:j0 + TILE])
                a1 = apool.tile([128, TILE], F32R, tag="a1")
                nc.sync.dma_start(a1[:], actT[128:256, j0:j0 + TILE])
                a2 = apool.tile([KC - 256, TILE], F32R, tag="a2")
                nc.sync.dma_start(a2[:], actT[256:KC, j0:j0 + TILE])
                avs = [a0, a1, a2]

                hp = ph.tile([128, 2 * TILE], F32, tag="hp")
                for m in range(2):
                    dst = hp[:, m * TILE:(m + 1) * TILE]
                    for c in range(3):
                        nc.tensor.matmul(
                            dst,
                            wcs[c][:, m * 128:(m + 1) * 128],
                            avs[c][:],
                            start=(c == 0),
                            stop=(c == 2),
                        )

                hn = hpool.tile([128, 2 * TILE], F32R, tag="hn")
                nc.scalar.activation(hn[:], hp[:], Tanh)
                nc.sync.dma_start(
                    houtT[:, :].rearrange("(m p) j -> p m j", m=2)
                        [:, :, j0:j0 + TILE],
                    hn[:].rearrange("p (m j) -> p m j", m=2),
                )

                lg = pl.tile([NOPS, TILE], F32, tag="lg")
                nc.tensor.matmul(lg[:], wp0[:], hn[:, 0:TILE],
                                 start=True, stop=False)
                nc.tensor.matmul(lg[:], wp1[:], hn[:, TILE:2 * TILE],
                                 start=False, stop=True)

                ex = spool.tile([NOPS, TILE], F32R, tag="ex")
                nc.scalar.activation(ex[:], lg[:], Exp, bias=bpt[:])

                sm = ps.tile([1, TILE], F32, tag="sm")
                nc.tensor.matmul(sm[:], ones[:, 0:1], ex[:],
                                 start=True, stop=True)
                rc = spool.tile([1, TILE], F32, tag="rc")
                nc.vector.reciprocal_approx_fast(rc[:], sm[:])
                bc = spool.tile([NOPS, TILE], F32, tag="bc")
                nc.gpsimd.partition_broadcast(bc[:], rc[:])

                ot = spool.tile([NOPS, TILE], F32, tag="ot")
                nc.vector.tensor_mul(ot[:], ex[:].bitcast(F32), bc[:])
                nc.sync.dma_start(ooutT[:, j0:j0 + TILE], ot[:])

    nc.finalize()
    return nc


def kernel(x, hidden, W_ih, b_ih, W_hh, b_hh, W_proj, b_proj):
    global _NC_CACHE, LAST_RESULTS, _LAST_IN_MAPS
    x = np.ascontiguousarray(np.asarray(x, dtype=np.float32))
    hidden = np.asarray(hidden, dtype=np.float32)
    W_ih = np.asarray(W_ih, dtype=np.float32)
    b_ih = np.asarray(b_ih, dtype=np.float32)
    W_hh = np.asarray(W_hh, dtype=np.float32)
    b_hh = np.asarray(b_hh, dtype=np.float32)
    W_proj = np.asarray(W_proj, dtype=np.float32)
    b_proj = np.asarray(b_proj, dtype=np.float32)

    wcatT = np.empty((KC, H), dtype=np.float32)
    wcatT[0:H] = W_hh.T
    wcatT[H:H + IN] = W_ih.T
    wcatT[H + IN] = b_ih + b_hh
    wpT = np.ascontiguousarray(W_proj.T)
    bp = np.ascontiguousarray(b_proj.reshape(NOPS, 1))
    ones12 = np.ones((NOPS, NOPS), dtype=np.float32)

    hiddenT = hidden.T  # [H, N] view; per-core column slices copied below
    xT = x.T            # [IN, N] view

    in_maps = []
    for c in range(NCORES):
        n0, n1 = c * NLOC, (c + 1) * NLOC
        actT = np.empty((KC, NLOC), dtype=np.float32)
        actT[0:H] = hiddenT[:, n0:n1]
        actT[H:H + IN] = xT[:, n0:n1]
        actT[H + IN] = 1.0
        in_maps.append({
            "actT": actT,
            "wcatT": wcatT,
            "wpT": wpT,
            "bp": bp,
            "ones12": ones12,
        })

    _LAST_IN_MAPS = in_maps
    if _NC_CACHE is None:
        _NC_CACHE = _build_nc()
    nc = _NC_CACHE

    res = run_bass_kernel_spmd(nc, in_maps, core_ids=list(range(NCORES)),
                               trace=TRACE)
    LAST_RESULTS = res

    out = np.empty((N, NOPS), dtype=np.float32)
    h_new = np.empty((N, H), dtype=np.float32)
    for c in range(NCORES):
        n0, n1 = c * NLOC, (c + 1) * NLOC
        out[n0:n1] = res.results[c]["ooutT"].T
        h_new[n0:n1] = res.results[c]["houtT"].T
    return out, h_new


# revision 7
# speedup vs baseline: 8.5702x; 8.5702x over previous
"""Trainium2 Bass kernel for a single-step RNN cell + softmax projection.

    h_new = tanh(x @ W_ih.T + b_ih + hidden @ W_hh.T + b_hh)   [N, 256]
    out   = softmax(h_new @ W_proj.T + b_proj, axis=1)          [N, 12]

Strategy: pure data parallel over the batch (N=524288) across 8 NeuronCores.
On-chip everything is computed in transposed orientation [feature, batch]
so the batch is the matmul moving dimension (512-column compute tiles,
2048-column DMA super-tiles):

    h_preT = W_cat.T^T @ actT       actT = [hidden.T ; x.T ; ones] (281 rows)
    (the ones row x b_h row folds both biases into the accumulation)
    h_newT = tanh(h_preT)                              -> stored transposed
    logitT = W_proj.T^T @ h_newT   (+ b_proj via Exp's per-partition bias)
    expT   = exp(logitT + b_proj)
    den    = ones.T @ expT          (partition-dim reduction via PE)
    outT   = expT * (ones @ (1/den))  (partition broadcast via fp32 matmul)

Matmuls run in float32r (full-rate fp32 on the PE). The host transposes
inputs while sharding and transposes outputs while gathering; all device
DMA transfers are fully contiguous.
"""

import numpy as np

import concourse.bacc as bacc
import concourse.bass as bass
import concourse.mybir as mybir
import concourse.tile as tile
from concourse.bass_utils import run_bass_kernel_spmd

F32 = mybir.dt.float32
F32R = mybir.dt.float32r

N = 524288
IN = 24
H = 256
NOPS = 12
NCORES = 8
NLOC = N // NCORES          # 65536 per core
TILE = 512                  # batch columns per compute tile
SUPER = 2048                # batch columns per DMA super-tile
NSUB = SUPER // TILE        # compute tiles per super-tile
NSUPER = NLOC // SUPER      # super-tiles per core
KC = H + IN + 1             # 281 contraction rows (hiddenT ; xT ; ones)

Tanh = mybir.ActivationFunctionType.Tanh
Exp = mybir.ActivationFunctionType.Exp

TRACE = False
LAST_RESULTS = None
_LAST_IN_MAPS = None

_NC_CACHE = None


def _build_nc(repeat=1):
    nc = bacc.Bacc("TRN2", target_bir_lowering=False, debug=False,
                   num_devices=NCORES)

    actT = nc.dram_tensor("actT", [KC, NLOC], F32R, kind="ExternalInput")
    wcatT = nc.dram_tensor("wcatT", [KC, H], F32R, kind="ExternalInput")
    wpT = nc.dram_tensor("wpT", [H, NOPS], F32R, kind="ExternalInput")
    bp = nc.dram_tensor("bp", [NOPS, 1], F32, kind="ExternalInput")
    ones12 = nc.dram_tensor("ones12", [NOPS, NOPS], F32R, kind="ExternalInput")
    houtT = nc.dram_tensor("houtT", [H, NLOC], F32R, kind="ExternalOutput")
    ooutT = nc.dram_tensor("ooutT", [NOPS, NLOC], F32, kind="ExternalOutput")

    # DRAM h_newT viewed as [p, m, col] for the m-major store of hn tiles
    houtT_r = houtT[:, :].rearrange("(m p) j -> p m j", m=2)

    with tile.TileContext(nc) as tc:
        with (
            tc.tile_pool(name="weights", bufs=1) as wpool,
            tc.tile_pool(name="acts", bufs=3) as apool,
            tc.tile_pool(name="hnew", bufs=2) as hpool,
            tc.tile_pool(name="soft", bufs=3) as spool,
            tc.tile_pool(name="osup", bufs=2) as opool,
            tc.tile_pool(name="psum_h", bufs=2, space="PSUM") as ph,
            tc.tile_pool(name="psum_l", bufs=2, space="PSUM") as pl,
            tc.tile_pool(name="psum_s", bufs=1, space="PSUM") as ps,
            tc.tile_pool(name="psum_b", bufs=1, space="PSUM") as pb,
        ):
            wc0 = wpool.tile([128, H], F32R, tag="wc0")
            nc.sync.dma_start(wc0[:], wcatT[0:128, :])
            wc1 = wpool.tile([128, H], F32R, tag="wc1")
            nc.sync.dma_start(wc1[:], wcatT[128:256, :])
            wc2 = wpool.tile([KC - 256, H], F32R, tag="wc2")
            nc.sync.dma_start(wc2[:], wcatT[256:KC, :])
            wp0 = wpool.tile([128, NOPS], F32R, tag="wp0")
            nc.sync.dma_start(wp0[:], wpT[0:128, :])
            wp1 = wpool.tile([128, NOPS], F32R, tag="wp1")
            nc.sync.dma_start(wp1[:], wpT[128:256, :])
            bpt = wpool.tile([NOPS, 1], F32, tag="bpt")
            nc.sync.dma_start(bpt[:], bp[:])
            ones = wpool.tile([NOPS, NOPS], F32R, tag="ones")
            nc.sync.dma_start(ones[:], ones12[:])
            ones_f = ones[0:1, :].bitcast(F32)

            wcs = [wc0, wc1, wc2]
            for _ in range(repeat):
                for s in range(NSUPER):
                    s0 = s * SUPER
                    a0 = apool.tile([128, SUPER], F32R, tag="a0")
                    nc.sync.dma_start(a0[:], actT[0:128, s0:s0 + SUPER])
                    a1 = apool.tile([128, SUPER], F32R, tag="a1")
                    nc.sync.dma_start(a1[:], actT[128:256, s0:s0 + SUPER])
                    a2 = apool.tile([KC - 256, SUPER], F32R, tag="a2")
                    nc.sync.dma_start(a2[:], actT[256:KC, s0:s0 + SUPER])
                    avs = [a0, a1, a2]

                    # hn holds the super-tile's h_newT, m-major: [128, 2, SUPER]
                    hn = hpool.tile([128, 2 * SUPER], F32R, tag="hn")
                    ot = opool.tile([NOPS, SUPER], F32, tag="ot")

                    for j in range(NSUB):
                        c0 = j * TILE
                        hp = ph.tile([128, 2 * TILE], F32, tag="hp")
                        for m in range(2):
                            dst = hp[:, m * TILE:(m + 1) * TILE]
                            for c in range(3):
                                nc.tensor.matmul(
                                    dst,
                                    wcs[c][:, m * 128:(m + 1) * 128],
                                    avs[c][:, c0:c0 + TILE],
                                    start=(c == 0),
                                    stop=(c == 2),
                                )

                        hm = [hn[:, m * SUPER + c0:m * SUPER + c0 + TILE]
                              for m in range(2)]
                        nc.scalar.activation(hm[0], hp[:, 0:TILE], Tanh)
                        nc.scalar.activation(hm[1], hp[:, TILE:2 * TILE], Tanh)

                        lg = pl.tile([NOPS, TILE], F32, tag="lg")
                        nc.tensor.matmul(lg[:], wp0[:], hm[0],
                                         start=True, stop=False)
                        nc.tensor.matmul(lg[:], wp1[:], hm[1],
                                         start=False, stop=True)

                        ex = spool.tile([NOPS, TILE], F32R, tag="ex")
                        nc.scalar.activation(ex[:], lg[:], Exp, bias=bpt[:])

                        sm = ps.tile([1, TILE], F32, tag="sm")
                        nc.tensor.matmul(sm[:], ones[:, 0:1], ex[:],
                                         start=True, stop=True)
                        rc = spool.tile([1, TILE], F32, tag="rc")
                        nc.vector.reciprocal_approx_fast(rc[:], sm[:])
                        rcr = spool.tile([1, TILE], F32R, tag="rcr")
                        nc.vector.tensor_copy(rcr[:], rc[:])
                        bc = pb.tile([NOPS, TILE], F32, tag="bc")
                        nc.tensor.matmul(bc[:], ones[0:1, :], rcr[:],
                                         start=True, stop=True)

                        nc.vector.tensor_mul(ot[:, c0:c0 + TILE],
                                             ex[:].bitcast(F32), bc[:])

                    nc.sync.dma_start(
                        houtT_r[:, :, s0:s0 + SUPER],
                        hn[:].rearrange("p (m j) -> p m j", m=2),
                    )
                    nc.sync.dma_start(ooutT[:, s0:s0 + SUPER], ot[:])

    nc.finalize()
    return nc


def kernel(x, hidden, W_ih, b_ih, W_hh, b_hh, W_proj, b_proj):
    global _NC_CACHE, LAST_RESULTS, _LAST_IN_MAPS
    x = np.ascontiguousarray(np.asarray(x, dtype=np.float32))
    hidden = np.asarray(hidden, dtype=np.float32)
    W_ih = np.asarray(W_ih, dtype=np.float32)
    b_ih = np.asarray(b_ih, dtype=np.float32)
    W_hh = np.asarray(W_hh, dtype=np.float32)
    b_hh = np.asarray(b_hh, dtype=np.float32)
    W_proj = np.asarray(W_proj, dtype=np.float32)
    b_proj = np.asarray(b_proj, dtype=np.float32)

    wcatT = np.empty((KC, H), dtype=np.float32)
    wcatT[0:H] = W_hh.T
    wcatT[H:H + IN] = W_ih.T
    wcatT[H + IN] = b_ih + b_hh
    wpT = np.ascontiguousarray(W_proj.T)
    bp = np.ascontiguousarray(b_proj.reshape(NOPS, 1))
    ones12 = np.ones((NOPS, NOPS), dtype=np.float32)

    hiddenT = hidden.T  # [H, N] view; per-core column slices copied below
    xT = x.T            # [IN, N] view

    in_maps = []
    for c in range(NCORES):
        n0, n1 = c * NLOC, (c + 1) * NLOC
        actT = np.empty((KC, NLOC), dtype=np.float32)
        actT[0:H] = hiddenT[:, n0:n1]
        actT[H:H + IN] = xT[:, n0:n1]
        actT[H + IN] = 1.0
        in_maps.append({
            "actT": actT,
            "wcatT": wcatT,
            "wpT": wpT,
            "bp": bp,
            "ones12": ones12,
        })

    _LAST_IN_MAPS = in_maps
    if _NC_CACHE is None:
        _NC_CACHE = _build_nc()
    nc = _NC_CACHE

    res = run_bass_kernel_spmd(nc, in_maps, core_ids=list(range(NCORES)),
                               trace=TRACE)
    LAST_RESULTS = res

    out = np.empty((N, NOPS), dtype=np.float32)
    h_new = np.empty((N, H), dtype=np.float32)
    for c in range(NCORES):
        n0, n1 = c * NLOC, (c + 1) * NLOC
        out[n0:n1] = res.results[c]["ooutT"].T
        h_new[n0:n1] = res.results[c]["houtT"].T
    return out, h_new
